# revision 6
# baseline (speedup 1.0000x reference)
"""ParabolicPool2D v3: max-plus pooling, 8 trn2 cores, batch-parallel.

out[b,c,ho,wo] = max_{ki,kj} f[b,c,2ho+ki-3,2wo+kj-3] + h[c,ki,kj],
h[c,ki,kj] = a[c,ki] + a[c,kj],  a = -z^2/(4t),  z = linspace(-2,3,7).

Separable two-stage (W then H). v3 changes vs v2:
- Host pre-casts f to fp16 and pre-deinterleaves even/odd columns into
  padded fe/fo tensors (pad rows AND cols at -30000): input DMA bytes
  halve, no device memsets, no ACT deinterleave, uniform full-rect slab
  DMAs.
- Biased-max taps run as TS-add (4x fp16 fast mode) + TT-max (2x) on DVE,
  with a configurable subset of taps' bias-adds offloaded to the ACT
  engine (activation Identity with per-partition bias) to balance engines;
  remaining taps can use 1-op STT (1x) where that wins.

Sharding: batch-parallel, 2 images per core, (b,c) dim 192; each image's H
split into two halves -> 384 half-images = 3 x 128 partition passes.
Half 0 covers out rows [0,56), local g rows r use padded-f row r;
half 1 covers out rows [56,112), local g rows r use padded-f row r+112.
"""

import os
import sys

sys.path.insert(0, "/opt/trn_rl_repo")

import numpy as np

from contextlib import ExitStack

from concourse import bacc, mybir, tile
from concourse.bass_utils import run_bass_kernel_spmd

KS = 7
C = 96
B = 16
H = 224
W = 224
HO = 112
WO = 112
NCORES = 8
BC = (B // NCORES) * C  # 192
R = 117  # g rows per half
HP = 230  # padded rows: 3 + 224 + 3
NEG = -30000.0

# stage-1 taps k=1..6: (k, parity, offset); src = (fe if parity else fo)[off:off+112]
S1_TAPS = [
    (1, 1, 0),
    (2, 0, 1),
    (3, 1, 1),
    (4, 0, 2),
    (5, 1, 2),
    (6, 0, 3),
]

PASSES = [
    [(0, 128, 0, 0)],
    [(0, 64, 0, 128), (64, 128, 1, 0)],
    [(0, 128, 1, 64)],
]
HALF_ROW0 = {0: 0, 1: 112}  # padded-f row of local row 0
HALF_HO0 = {0: 0, 1: 56}

SLABS = [(0, 30), (30, 59), (59, 88), (88, 117)]
SLABS2 = [(0, 59), (59, 117)]

# per-tap execution plan for the 6 non-init taps of each stage:
#   "stt"  - one scalar_tensor_tensor on DVE (1x)
#   "dve"  - TS-add tmp on DVE (4x) + TT-max merge on DVE (2x)
#   "act"  - activation-Identity-bias tmp on ACT + TT-max merge on DVE
S1_PLAN = ["act", "dve", "act", "dve", "act", "dve"]
S2_PLAN = ["act", "dve", "act", "dve", "act", "dve"]

_CACHE = {}


def _build(iters=1, s1_plan=None, s2_plan=None, init_plan="dve", slabs=None):
    s1_plan = s1_plan or S1_PLAN
    s2_plan = s2_plan or S2_PLAN
    slabs = slabs or SLABS2
    slab_rows = max(r1 - r0 for r0, r1 in slabs)
    nc = bacc.Bacc("TRN2", target_bir_lowering=False, debug=False)
    f32 = mybir.dt.float32
    f16 = mybir.dt.float16
    fe_d = nc.dram_tensor("fe", [BC, HP, 114], f16, kind="ExternalInput")
    fo_d = nc.dram_tensor("fo", [BC, HP, 115], f16, kind="ExternalInput")
    bias_d = nc.dram_tensor("bias", [len(PASSES), 128, KS], f32, kind="ExternalInput")
    out_d = nc.dram_tensor("out", [BC, HO, WO], f16, kind="ExternalOutput")
    fea, foa, ba, oa = fe_d.ap(), fo_d.ap(), bias_d.ap(), out_d.ap()

    add, mx = mybir.AluOpType.add, mybir.AluOpType.max
    ident = mybir.ActivationFunctionType.Identity

    with ExitStack() as ctx:
        tc = ctx.enter_context(tile.TileContext(nc))
        fin_pool = ctx.enter_context(tc.tile_pool(name="fin", bufs=3))
        tmp_pool = ctx.enter_context(tc.tile_pool(name="tmp", bufs=3))
        g_pool = ctx.enter_context(tc.tile_pool(name="g", bufs=2))
        out_pool = ctx.enter_context(tc.tile_pool(name="outp", bufs=2))
        bias_pool = ctx.enter_context(tc.tile_pool(name="bias", bufs=2))

        for t, groups in [(t, g) for _ in range(iters) for t, g in enumerate(PASSES)]:
            bias_t = bias_pool.tile([128, KS], f32)
            nc.sync.dma_start(bias_t[:], ba[t])
            bias16 = bias_pool.tile([128, KS], f16)
            nc.scalar.copy(bias16[:], bias_t[:])
            g = g_pool.tile([128, R, WO], f16)

            for rl0, rl1 in slabs:
                rs = rl1 - rl0
                fe = fin_pool.tile([128, slab_rows, 114], f16)
                fo = fin_pool.tile([128, slab_rows, 115], f16)
                for p0, p1, half, bc0 in groups:
                    r0 = HALF_ROW0[half] + rl0
                    nc.sync.dma_start(
                        fe[p0:p1, 0:rs, :], fea[bc0 : bc0 + (p1 - p0), r0 : r0 + rs, :]
                    )
                    nc.sync.dma_start(
                        fo[p0:p1, 0:rs, :], foa[bc0 : bc0 + (p1 - p0), r0 : r0 + rs, :]
                    )

                gs = g[:, rl0:rl1, :]
                # init tap k=0: g = fo[wo] + a0
                if init_plan == "act":
                    nc.scalar.activation(
                        gs, fo[:, 0:rs, 0:112], ident, bias=bias_t[:, 0:1]
                    )
                else:
                    nc.vector.tensor_scalar_add(
                        gs, fo[:, 0:rs, 0:112], bias_t[:, 0:1]
                    )
                for (k, par, off), plan in zip(S1_TAPS, s1_plan):
                    src = (fe if par else fo)[:, 0:rs, off : off + 112]
                    if plan == "stt":
                        nc.vector.scalar_tensor_tensor(
                            gs, src, bias16[:, k : k + 1], gs, add, mx
                        )
                    else:
                        tmp = tmp_pool.tile([128, slab_rows, 112], f16)
                        tv = tmp[:, 0:rs, :]
                        if plan == "dve":
                            nc.vector.tensor_scalar_add(tv, src, bias_t[:, k : k + 1])
                        else:
                            nc.scalar.activation(
                                tv, src, ident, bias=bias_t[:, k : k + 1]
                            )
                        nc.vector.tensor_tensor(gs, tv, gs, mx)

            out_t = out_pool.tile([128, 56, WO], f16)
            if init_plan == "act":
                nc.scalar.activation(
                    out_t[:], g[:, 0:111:2, :], ident, bias=bias_t[:, 0:1]
                )
            else:
                nc.vector.tensor_scalar_add(
                    out_t[:], g[:, 0:111:2, :], bias_t[:, 0:1]
                )
            for k, plan in zip((1, 2, 3, 4, 5, 6), s2_plan):
                src = g[:, k : k + 111 : 2, :]
                if plan == "stt":
                    nc.vector.scalar_tensor_tensor(
                        out_t[:], src, bias16[:, k : k + 1], out_t[:], add, mx
                    )
                else:
                    tmp = tmp_pool.tile([128, 56, 112], f16)
                    if plan == "dve":
                        nc.vector.tensor_scalar_add(tmp[:], src, bias_t[:, k : k + 1])
                    else:
                        nc.scalar.activation(
                            tmp[:], src, ident, bias=bias_t[:, k : k + 1]
                        )
                    nc.vector.tensor_tensor(out_t[:], tmp[:], out_t[:], mx)
            for p0, p1, half, bc0 in groups:
                ho0 = HALF_HO0[half]
                nc.sync.dma_start(
                    oa[bc0 : bc0 + (p1 - p0), ho0 : ho0 + 56, :],
                    out_t[p0:p1, :, :],
                )
    nc.compile()
    return nc


def _bias_array(t: np.ndarray) -> np.ndarray:
    z = np.linspace(-2.0, 3.0, KS, dtype=np.float32)
    a = -(z[None, :] ** 2) / (4.0 * t[:, None].astype(np.float32))  # [C, KS]
    a_bc = np.tile(a, (B // NCORES, 1))  # [192, KS]
    out = np.empty((len(PASSES), 128, KS), dtype=np.float32)
    for t_i, groups in enumerate(PASSES):
        for p0, p1, _half, bc0 in groups:
            out[t_i, p0:p1] = a_bc[bc0 : bc0 + (p1 - p0)]
    return out


def _prep_inputs(f: np.ndarray, t: np.ndarray):
    """Host-side: fp16 cast, column deinterleave, pad rows+cols with NEG."""
    bias = _bias_array(np.asarray(t))
    f16 = np.asarray(f, dtype=np.float16).reshape(B * C, H, W)
    per_core = B // NCORES
    fe = np.full((B * C, HP, 114), NEG, dtype=np.float16)
    fo = np.full((B * C, HP, 115), NEG, dtype=np.float16)
    # fe[j] = f[2j-2] for j in 1..112; fo[j] = f[2j-3] for j in 2..113
    fe[:, 3 : 3 + H, 1:113] = f16[:, :, 0::2]
    fo[:, 3 : 3 + H, 2:114] = f16[:, :, 1::2]
    in_maps = [
        {
            "fe": np.ascontiguousarray(
                fe[s * per_core * C : (s + 1) * per_core * C]
            ),
            "fo": np.ascontiguousarray(
                fo[s * per_core * C : (s + 1) * per_core * C]
            ),
            "bias": bias,
        }
        for s in range(NCORES)
    ]
    return in_maps


LAST_EXEC_NS = None


def _make_runner(nc):
    import jax
    from jax.experimental.shard_map import shard_map
    from jax.sharding import Mesh, NamedSharding, PartitionSpec

    from concourse import bass2jax, mybir as _mybir

    bass2jax.install_neuronx_cc_hook()
    partition_name = nc.partition_id_tensor.name if nc.partition_id_tensor else None
    in_names, out_names, out_avals = [], [], []
    for alloc in nc.m.functions[0].allocations:
        if not isinstance(alloc, _mybir.MemoryLocationSet):
            continue
        name = alloc.memorylocations[0].name
        if alloc.kind == "ExternalInput":
            if name != partition_name:
                in_names.append(name)
        elif alloc.kind == "ExternalOutput":
            out_names.append(name)
            out_avals.append(
                jax.core.ShapedArray(
                    tuple(alloc.tensor_shape), _mybir.dt.np(alloc.dtype)
                )
            )
    n_params, n_outs = len(in_names), len(out_avals)
    all_names = list(in_names + out_names)
    if partition_name is not None:
        all_names.append(partition_name)
    all_names = tuple(all_names)
    donate = tuple(range(n_params, n_params + n_outs))

    def _body(*args):
        operands = list(args)
        if partition_name is not None:
            operands.append(bass2jax.partition_id_tensor())
        return tuple(
            bass2jax._bass_exec_p.bind(
                *operands,
                out_avals=tuple(out_avals),
                in_names=all_names,
                out_names=tuple(out_names),
                lowering_input_output_aliases=(),
                sim_require_finite=True,
                sim_require_nnan=True,
                nc=nc,
            )
        )

    mesh = Mesh(np.asarray(jax.devices()[:NCORES]), ("core",))
    sharded = jax.jit(
        shard_map(
            _body,
            mesh=mesh,
            in_specs=(PartitionSpec("core"),) * (n_params + n_outs),
            out_specs=(PartitionSpec("core"),) * n_outs,
            check_rep=False,
        ),
        donate_argnums=donate,
        keep_unused=True,
    )
    sh = NamedSharding(mesh, PartitionSpec("core"))
    return sharded, in_names, out_names, out_avals, sh


def _timed_run(nc, in_maps, ncalls=8):
    import time as _time

    import jax

    sharded, in_names, out_names, out_avals, sh = _make_runner(nc)
    concat_in = [
        np.concatenate([np.asarray(m[nm]) for m in in_maps], axis=0)
        for nm in in_names
    ]
    dev_in = [jax.device_put(x, sh) for x in concat_in]
    zero_sets = [
        [
            jax.device_put(
                np.zeros((NCORES * a.shape[0], *a.shape[1:]), a.dtype), sh
            )
            for a in out_avals
        ]
        for _ in range(ncalls + 1)
    ]
    out = sharded(*dev_in, *zero_sets[0])
    jax.block_until_ready(out)
    times = []
    for i in range(1, ncalls + 1):
        t0 = _time.perf_counter()
        out = sharded(*dev_in, *zero_sets[i])
        jax.block_until_ready(out)
        times.append(_time.perf_counter() - t0)
    outs = [
        {
            nm: np.asarray(out[i]).reshape(NCORES, *out_avals[i].shape)[c]
            for i, nm in enumerate(out_names)
        }
        for c in range(NCORES)
    ]
    return times, outs


def measure_hw_time(f: np.ndarray, t: np.ndarray, iters=25, ncalls=12):
    """Per-iteration HW time via N-iteration differencing; the 1-iter and
    N-iter builds are called interleaved so slow session drift cancels in
    the per-pair deltas."""
    global LAST_EXEC_NS
    import time as _time

    import jax

    in_maps = _prep_inputs(f, t)
    runners = []
    for it in (1, iters):
        nc = _build(it)
        sharded, in_names, out_names, out_avals, sh = _make_runner(nc)
        concat_in = [
            np.concatenate([np.asarray(m[nm]) for m in in_maps], axis=0)
            for nm in in_names
        ]
        dev_in = [jax.device_put(x, sh) for x in concat_in]
        runners.append((sharded, dev_in, out_avals, sh))

    def call(idx):
        sharded, dev_in, out_avals, sh = runners[idx]
        zeros = [
            jax.device_put(
                np.zeros((NCORES * a.shape[0], *a.shape[1:]), a.dtype), sh
            )
            for a in out_avals
        ]
        t0 = _time.perf_counter()
        out = sharded(*dev_in, *zeros)
        jax.block_until_ready(out)
        return _time.perf_counter() - t0

    call(0)
    call(1)
    t1, tN = [], []
    hw_ns = None
    for _attempt in range(3):
        for _ in range(max(ncalls, 12)):
            t1.append(call(0))
            tN.append(call(1))
        # Heavy right-tail contamination (shared device): the smallest calls
        # hit uncontended windows; estimate from the lowest samples.
        k = max(3, len(t1) // 6)
        m1 = float(np.median(sorted(t1)[:k]))
        mN = float(np.median(sorted(tN)[:k]))
        hw_ns = (mN - m1) / (iters - 1) * 1e9
        if hw_ns > 0:
            break
    LAST_EXEC_NS = int(hw_ns)
    return {
        "t1": t1,
        "tN": tN,
        "iters": iters,
        "hw_ns": hw_ns,
        "upper_bound_ns": min(t1) * 1e9,
    }


def kernel(f: np.ndarray, t: np.ndarray) -> np.ndarray:
    global LAST_EXEC_NS
    if "nc" not in _CACHE:
        _CACHE["nc"] = _build()
    nc = _CACHE["nc"]

    in_maps = _prep_inputs(f, t)
    trace = os.environ.get("BASS_TRACE", "0") == "1"
    res = run_bass_kernel_spmd(nc, in_maps, core_ids=list(range(NCORES)), trace=trace)
    LAST_EXEC_NS = res.exec_time_ns

    per_core = B // NCORES
    out = np.empty((B, C, HO, WO), dtype=np.float32)
    for s in range(NCORES):
        out[s * per_core : (s + 1) * per_core] = res.results[s]["out"].reshape(
            per_core, C, HO, WO
        )
    return out


# revision 9
# speedup vs baseline: 4.2046x; 4.2046x over previous
"""ParabolicPool2D v3: max-plus pooling, 8 trn2 cores, batch-parallel.

out[b,c,ho,wo] = max_{ki,kj} f[b,c,2ho+ki-3,2wo+kj-3] + h[c,ki,kj],
h[c,ki,kj] = a[c,ki] + a[c,kj],  a = -z^2/(4t),  z = linspace(-2,3,7).

Separable two-stage (W then H). v3 changes vs v2:
- Host pre-casts f to fp16 and pre-deinterleaves even/odd columns into
  padded fe/fo tensors (pad rows AND cols at -30000): input DMA bytes
  halve, no device memsets, no ACT deinterleave, uniform full-rect slab
  DMAs.
- Biased-max taps run as TS-add (4x fp16 fast mode) + TT-max (2x) on DVE,
  with a configurable subset of taps' bias-adds offloaded to the ACT
  engine (activation Identity with per-partition bias) to balance engines;
  remaining taps can use 1-op STT (1x) where that wins.

Sharding: batch-parallel, 2 images per core, (b,c) dim 192; each image's H
split into two halves -> 384 half-images = 3 x 128 partition passes.
Half 0 covers out rows [0,56), local g rows r use padded-f row r;
half 1 covers out rows [56,112), local g rows r use padded-f row r+112.
"""

import os
import sys

sys.path.insert(0, "/opt/trn_rl_repo")

import numpy as np

from contextlib import ExitStack

from concourse import bacc, mybir, tile
from concourse.bass_utils import run_bass_kernel_spmd

KS = 7
C = 96
B = 16
H = 224
W = 224
HO = 112
WO = 112
NCORES = 8
BC = (B // NCORES) * C  # 192
R = 117  # g rows per half
HP = 230  # padded rows: 3 + 224 + 3
NEG = -30000.0

# stage-1 taps k=1..6: (k, parity, offset); src = (fe if parity else fo)[off:off+112]
S1_TAPS = [
    (1, 1, 0),
    (2, 0, 1),
    (3, 1, 1),
    (4, 0, 2),
    (5, 1, 2),
    (6, 0, 3),
]

PASSES = [
    [(0, 128, 0, 0)],
    [(0, 64, 0, 128), (64, 128, 1, 0)],
    [(0, 128, 1, 64)],
]
HALF_ROW0 = {0: 0, 1: 112}  # padded-f row of local row 0
HALF_HO0 = {0: 0, 1: 56}

SLABS = [(0, 30), (30, 59), (59, 88), (88, 117)]
SLABS2 = [(0, 59), (59, 117)]

# per-tap execution plan for the 6 non-init taps of each stage:
#   "stt"  - one scalar_tensor_tensor on DVE (1x)
#   "dve"  - TS-add tmp on DVE (4x) + TT-max merge on DVE (2x)
#   "act"  - activation-Identity-bias tmp on ACT + TT-max merge on DVE
S1_PLAN = ["act", "dve", "act", "dve", "act", "dve"]
S2_PLAN = ["act", "dve", "act", "dve", "act", "dve"]

_CACHE = {}


def _build(iters=1, s1_plan=None, s2_plan=None, init_plan="dve", slabs=None,
           fin_bufs=5, tmp_bufs=5):
    s1_plan = s1_plan or S1_PLAN
    s2_plan = s2_plan or S2_PLAN
    slabs = slabs or SLABS
    slab_rows = max(r1 - r0 for r0, r1 in slabs)
    nc = bacc.Bacc("TRN2", target_bir_lowering=False, debug=False)
    f32 = mybir.dt.float32
    f16 = mybir.dt.float16
    fe_d = nc.dram_tensor("fe", [BC, HP, 114], f16, kind="ExternalInput")
    fo_d = nc.dram_tensor("fo", [BC, HP, 115], f16, kind="ExternalInput")
    bias_d = nc.dram_tensor("bias", [len(PASSES), 128, KS], f32, kind="ExternalInput")
    out_d = nc.dram_tensor("out", [BC, HO, WO], f16, kind="ExternalOutput")
    fea, foa, ba, oa = fe_d.ap(), fo_d.ap(), bias_d.ap(), out_d.ap()

    add, mx = mybir.AluOpType.add, mybir.AluOpType.max
    ident = mybir.ActivationFunctionType.Identity

    with ExitStack() as ctx:
        tc = ctx.enter_context(tile.TileContext(nc))
        fin_pool = ctx.enter_context(tc.tile_pool(name="fin", bufs=fin_bufs))
        tmp_pool = ctx.enter_context(tc.tile_pool(name="tmp", bufs=tmp_bufs))
        g_pool = ctx.enter_context(tc.tile_pool(name="g", bufs=2))
        out_pool = ctx.enter_context(tc.tile_pool(name="outp", bufs=2))
        bias_pool = ctx.enter_context(tc.tile_pool(name="bias", bufs=2))

        for t, groups in [(t, g) for _ in range(iters) for t, g in enumerate(PASSES)]:
            bias_t = bias_pool.tile([128, KS], f32)
            nc.sync.dma_start(bias_t[:], ba[t])
            bias16 = bias_pool.tile([128, KS], f16)
            nc.scalar.copy(bias16[:], bias_t[:])
            g = g_pool.tile([128, R, WO], f16)

            for rl0, rl1 in slabs:
                rs = rl1 - rl0
                fe = fin_pool.tile([128, slab_rows, 114], f16)
                fo = fin_pool.tile([128, slab_rows, 115], f16)
                for p0, p1, half, bc0 in groups:
                    r0 = HALF_ROW0[half] + rl0
                    nc.sync.dma_start(
                        fe[p0:p1, 0:rs, :], fea[bc0 : bc0 + (p1 - p0), r0 : r0 + rs, :]
                    )
                    nc.sync.dma_start(
                        fo[p0:p1, 0:rs, :], foa[bc0 : bc0 + (p1 - p0), r0 : r0 + rs, :]
                    )

                gs = g[:, rl0:rl1, :]
                # init tap k=0: g = fo[wo] + a0
                if init_plan == "act":
                    nc.scalar.activation(
                        gs, fo[:, 0:rs, 0:112], ident, bias=bias_t[:, 0:1]
                    )
                else:
                    nc.vector.tensor_scalar_add(
                        gs, fo[:, 0:rs, 0:112], bias_t[:, 0:1]
                    )
                for (k, par, off), plan in zip(S1_TAPS, s1_plan):
                    src = (fe if par else fo)[:, 0:rs, off : off + 112]
                    if plan == "stt":
                        nc.vector.scalar_tensor_tensor(
                            gs, src, bias16[:, k : k + 1], gs, add, mx
                        )
                    else:
                        tmp = tmp_pool.tile([128, slab_rows, 112], f16)
                        tv = tmp[:, 0:rs, :]
                        if plan == "dve":
                            nc.vector.tensor_scalar_add(tv, src, bias_t[:, k : k + 1])
                        else:
                            nc.scalar.activation(
                                tv, src, ident, bias=bias_t[:, k : k + 1]
                            )
                        nc.vector.tensor_tensor(gs, tv, gs, mx)

            out_t = out_pool.tile([128, 56, WO], f16)
            if init_plan == "act":
                nc.scalar.activation(
                    out_t[:], g[:, 0:111:2, :], ident, bias=bias_t[:, 0:1]
                )
            else:
                nc.vector.tensor_scalar_add(
                    out_t[:], g[:, 0:111:2, :], bias_t[:, 0:1]
                )
            for k, plan in zip((1, 2, 3, 4, 5, 6), s2_plan):
                src = g[:, k : k + 111 : 2, :]
                if plan == "stt":
                    nc.vector.scalar_tensor_tensor(
                        out_t[:], src, bias16[:, k : k + 1], out_t[:], add, mx
                    )
                else:
                    tmp = tmp_pool.tile([128, 56, 112], f16)
                    if plan == "dve":
                        nc.vector.tensor_scalar_add(tmp[:], src, bias_t[:, k : k + 1])
                    else:
                        nc.scalar.activation(
                            tmp[:], src, ident, bias=bias_t[:, k : k + 1]
                        )
                    nc.vector.tensor_tensor(out_t[:], tmp[:], out_t[:], mx)
            for p0, p1, half, bc0 in groups:
                ho0 = HALF_HO0[half]
                nc.sync.dma_start(
                    oa[bc0 : bc0 + (p1 - p0), ho0 : ho0 + 56, :],
                    out_t[p0:p1, :, :],
                )
    nc.compile()
    return nc


def _bias_array(t: np.ndarray) -> np.ndarray:
    z = np.linspace(-2.0, 3.0, KS, dtype=np.float32)
    a = -(z[None, :] ** 2) / (4.0 * t[:, None].astype(np.float32))  # [C, KS]
    a_bc = np.tile(a, (B // NCORES, 1))  # [192, KS]
    out = np.empty((len(PASSES), 128, KS), dtype=np.float32)
    for t_i, groups in enumerate(PASSES):
        for p0, p1, _half, bc0 in groups:
            out[t_i, p0:p1] = a_bc[bc0 : bc0 + (p1 - p0)]
    return out


def _prep_inputs(f: np.ndarray, t: np.ndarray):
    """Host-side: fp16 cast, column deinterleave, pad rows+cols with NEG."""
    bias = _bias_array(np.asarray(t))
    f16 = np.asarray(f, dtype=np.float16).reshape(B * C, H, W)
    per_core = B // NCORES
    fe = np.full((B * C, HP, 114), NEG, dtype=np.float16)
    fo = np.full((B * C, HP, 115), NEG, dtype=np.float16)
    # fe[j] = f[2j-2] for j in 1..112; fo[j] = f[2j-3] for j in 2..113
    fe[:, 3 : 3 + H, 1:113] = f16[:, :, 0::2]
    fo[:, 3 : 3 + H, 2:114] = f16[:, :, 1::2]
    in_maps = [
        {
            "fe": np.ascontiguousarray(
                fe[s * per_core * C : (s + 1) * per_core * C]
            ),
            "fo": np.ascontiguousarray(
                fo[s * per_core * C : (s + 1) * per_core * C]
            ),
            "bias": bias,
        }
        for s in range(NCORES)
    ]
    return in_maps


LAST_EXEC_NS = None


def _make_runner(nc):
    import jax
    from jax.experimental.shard_map import shard_map
    from jax.sharding import Mesh, NamedSharding, PartitionSpec

    from concourse import bass2jax, mybir as _mybir

    bass2jax.install_neuronx_cc_hook()
    partition_name = nc.partition_id_tensor.name if nc.partition_id_tensor else None
    in_names, out_names, out_avals = [], [], []
    for alloc in nc.m.functions[0].allocations:
        if not isinstance(alloc, _mybir.MemoryLocationSet):
            continue
        name = alloc.memorylocations[0].name
        if alloc.kind == "ExternalInput":
            if name != partition_name:
                in_names.append(name)
        elif alloc.kind == "ExternalOutput":
            out_names.append(name)
            out_avals.append(
                jax.core.ShapedArray(
                    tuple(alloc.tensor_shape), _mybir.dt.np(alloc.dtype)
                )
            )
    n_params, n_outs = len(in_names), len(out_avals)
    all_names = list(in_names + out_names)
    if partition_name is not None:
        all_names.append(partition_name)
    all_names = tuple(all_names)
    donate = tuple(range(n_params, n_params + n_outs))

    def _body(*args):
        operands = list(args)
        if partition_name is not None:
            operands.append(bass2jax.partition_id_tensor())
        return tuple(
            bass2jax._bass_exec_p.bind(
                *operands,
                out_avals=tuple(out_avals),
                in_names=all_names,
                out_names=tuple(out_names),
                lowering_input_output_aliases=(),
                sim_require_finite=True,
                sim_require_nnan=True,
                nc=nc,
            )
        )

    mesh = Mesh(np.asarray(jax.devices()[:NCORES]), ("core",))
    sharded = jax.jit(
        shard_map(
            _body,
            mesh=mesh,
            in_specs=(PartitionSpec("core"),) * (n_params + n_outs),
            out_specs=(PartitionSpec("core"),) * n_outs,
            check_rep=False,
        ),
        donate_argnums=donate,
        keep_unused=True,
    )
    sh = NamedSharding(mesh, PartitionSpec("core"))
    return sharded, in_names, out_names, out_avals, sh


def _timed_run(nc, in_maps, ncalls=8):
    import time as _time

    import jax

    sharded, in_names, out_names, out_avals, sh = _make_runner(nc)
    concat_in = [
        np.concatenate([np.asarray(m[nm]) for m in in_maps], axis=0)
        for nm in in_names
    ]
    dev_in = [jax.device_put(x, sh) for x in concat_in]
    zero_sets = [
        [
            jax.device_put(
                np.zeros((NCORES * a.shape[0], *a.shape[1:]), a.dtype), sh
            )
            for a in out_avals
        ]
        for _ in range(ncalls + 1)
    ]
    out = sharded(*dev_in, *zero_sets[0])
    jax.block_until_ready(out)
    times = []
    for i in range(1, ncalls + 1):
        t0 = _time.perf_counter()
        out = sharded(*dev_in, *zero_sets[i])
        jax.block_until_ready(out)
        times.append(_time.perf_counter() - t0)
    outs = [
        {
            nm: np.asarray(out[i]).reshape(NCORES, *out_avals[i].shape)[c]
            for i, nm in enumerate(out_names)
        }
        for c in range(NCORES)
    ]
    return times, outs


def measure_hw_time(f: np.ndarray, t: np.ndarray, iters=25, ncalls=12):
    """Per-iteration HW time via N-iteration differencing; the 1-iter and
    N-iter builds are called interleaved so slow session drift cancels in
    the per-pair deltas."""
    global LAST_EXEC_NS
    import time as _time

    import jax

    in_maps = _prep_inputs(f, t)
    runners = []
    for it in (1, iters):
        nc = _build(it)
        sharded, in_names, out_names, out_avals, sh = _make_runner(nc)
        concat_in = [
            np.concatenate([np.asarray(m[nm]) for m in in_maps], axis=0)
            for nm in in_names
        ]
        dev_in = [jax.device_put(x, sh) for x in concat_in]
        runners.append((sharded, dev_in, out_avals, sh))

    def call(idx):
        sharded, dev_in, out_avals, sh = runners[idx]
        zeros = [
            jax.device_put(
                np.zeros((NCORES * a.shape[0], *a.shape[1:]), a.dtype), sh
            )
            for a in out_avals
        ]
        t0 = _time.perf_counter()
        out = sharded(*dev_in, *zeros)
        jax.block_until_ready(out)
        return _time.perf_counter() - t0

    call(0)
    call(1)
    t1, tN = [], []
    hw_ns = None
    for _attempt in range(3):
        for _ in range(max(ncalls, 12)):
            t1.append(call(0))
            tN.append(call(1))
        # Heavy right-tail contamination (shared device): the smallest calls
        # hit uncontended windows; estimate from the lowest samples.
        k = max(3, len(t1) // 6)
        m1 = float(np.median(sorted(t1)[:k]))
        mN = float(np.median(sorted(tN)[:k]))
        hw_ns = (mN - m1) / (iters - 1) * 1e9
        if hw_ns > 0:
            break
    LAST_EXEC_NS = int(hw_ns)
    return {
        "t1": t1,
        "tN": tN,
        "iters": iters,
        "hw_ns": hw_ns,
        "upper_bound_ns": min(t1) * 1e9,
    }


def kernel(f: np.ndarray, t: np.ndarray) -> np.ndarray:
    global LAST_EXEC_NS
    if "nc" not in _CACHE:
        _CACHE["nc"] = _build()
    nc = _CACHE["nc"]

    in_maps = _prep_inputs(f, t)
    trace = os.environ.get("BASS_TRACE", "0") == "1"
    res = run_bass_kernel_spmd(nc, in_maps, core_ids=list(range(NCORES)), trace=trace)
    LAST_EXEC_NS = res.exec_time_ns

    per_core = B // NCORES
    out = np.empty((B, C, HO, WO), dtype=np.float32)
    for s in range(NCORES):
        out[s * per_core : (s + 1) * per_core] = res.results[s]["out"].reshape(
            per_core, C, HO, WO
        )
    return out
